# revision 37
# baseline (speedup 1.0000x reference)
"""LIF (leaky integrate-and-fire) forward kernel for Trainium2, 8-core SPMD.

Reference semantics (per element, scan over T):
    u = 0.5*u + x_t
    o_t = (u - 1 >= 0) ? 1.0 : 0.0
    u = u - o_t

Sharding: pure data parallel over batch B=32 -> 4 batches per core.
Per-core shard: x [4, 16, 128, 1024] f32; C=128 on the SBUF partition dim,
(b, h*w) on the free dim -> a [128, 4096] tile per timestep, split into two
2048-column slices (batches 0-1 / 2-3) that pipeline independently:

    u' = (v * 0.5) + x_t         DVE scalar_tensor_tensor
    o  = Sign(u' - 1) -> uint8   ACT (saturating cast: -1 -> 0, so o=(u'>1);
                                 u'==1.0 exactly stores 0 where the ref
                                 fires - measure-zero on randn inputs)
    v  = (o * -1.0) + u'         DVE stt (u8 in0 casts at full speed)

Step 0 skips the integrate (u'(0) = x since v0 = 0, exact) and step T-1
skips the reset (v unused afterwards), so the DVE runs 60 stts total and
is >98% busy in steady state - the kernel is vector-engine bound at
~2.3us per [128, 2048] stt. ACT signs trail the DVE by one op; stores
launch per half-tile as soon as its Sign lands; x loads are four
per-batch transfers on parallel DMA queues, NX=4 deep.

Engines deliberately NOT used: GPSIMD (concurrent GPSIMD+DVE SBUF
traffic halves DVE throughput - measured, net negative), PE (fp32
matmul = 2 HW passes + LDWEIGHTS + p-state ramps => ~6ns/col vs DVE's
1.1ns/col - measured net negative at every CP tried).

All f32 arithmetic rounds identically to the jax reference, so the
output is bit-exact away from the u'==1.0 boundary.
"""

import numpy as np

B, T, C, HW = 32, 16, 128, 1024
NCORES = 8
BLOC = B // NCORES  # 4 batches per core
FREE = BLOC * HW    # 4096
NX = 5              # x buffer slots
NSL = 2             # column slices
WD = FREE // NSL    # 2048

_cached = {}


def _build_nc():
    import concourse.bass as bass
    import concourse.mybir as mybir
    from contextlib import ExitStack

    f32 = mybir.dt.float32
    u8 = mybir.dt.uint8
    Alu = mybir.AluOpType
    Act = mybir.ActivationFunctionType

    nc = bass.Bass()
    x_d = nc.declare_dram_parameter("x", [BLOC, T, C, HW], f32, isOutput=False)
    o_d = nc.declare_dram_parameter("o", [BLOC, T, C, HW], u8, isOutput=True)

    # counter values (1-based) after each instr:
    # ACT: 2 Signs per step
    def sa_sign(t, j):  # j in {1, 2}
        return 2 * t + j

    # DVE: 1 memset preamble; step 0: 2 resets; steps 1..14: 2 integrates +
    # 2 resets; step 15: 2 integrates
    def sv_u(t, j):  # u'Dj(t), t >= 1
        return 4 * t + j - 1

    def sv_r(t, j):  # resetDj(t), t <= 14
        return (1 + j) if t == 0 else (4 * t + 1 + j)

    def sv_all(t):  # everything of step t done
        if t == 0:
            return 3
        return 4 * t + (1 if t == T - 1 else 3)

    with ExitStack() as ctx:
        xt = [
            ctx.enter_context(nc.sbuf_tensor(f"xt{i}", [C, FREE], f32))
            for i in range(NX)
        ]
        ud = [
            ctx.enter_context(nc.sbuf_tensor(f"ud{i}", [C, FREE], f32))
            for i in range(2)
        ]
        ot = [
            ctx.enter_context(nc.sbuf_tensor(f"ot{i}", [C, FREE], u8))
            for i in range(2)
        ]
        vd = ctx.enter_context(nc.sbuf_tensor("vd", [C, FREE], f32))
        bm1 = ctx.enter_context(nc.sbuf_tensor("bm1", [C, 1], f32))

        s_x = [
            ctx.enter_context(nc.semaphore(f"s_x{j}")) for j in range(NSL)
        ]
        s_o = ctx.enter_context(nc.semaphore("s_o"))
        s_v = ctx.enter_context(nc.semaphore("s_v"))
        s_a = ctx.enter_context(nc.semaphore("s_a"))
        block = ctx.enter_context(nc.Block())

        def sl(buf, j):
            return buf[:, j * WD : (j + 1) * WD]

        @block.sync
        def _(sync: bass.BassEngine):
            def issue_x(t):
                # one transfer per batch, parallel DMA queues; slice j waits
                # only on its own two batches. Step 0 is latency-critical
                # (kernel ramp), so it goes out as 8 quarter-transfers.
                if t >= NX:
                    # xt slot free once step t-NX consumed it
                    if t - NX == 0:
                        sync.wait_ge(s_v, sv_all(0))
                        sync.wait_ge(s_a, sa_sign(0, 2))
                    else:
                        sync.wait_ge(s_v, sv_u(t - NX, 2))
                for b in range(BLOC):
                    if t == 0:
                        for h in range(2):
                            c0 = b * HW + h * (HW // 2)
                            sync.dma_start(
                                out=xt[0][:, c0 : c0 + HW // 2],
                                in_=x_d[b, 0][
                                    :, h * (HW // 2) : (h + 1) * (HW // 2)
                                ],
                            ).then_inc(s_x[b // 2], 16)
                    else:
                        sync.dma_start(
                            out=xt[t % NX][:, b * HW : (b + 1) * HW],
                            in_=x_d[b, t],
                        ).then_inc(s_x[b // 2], 16)

            issue_x(0)
            for t in range(T):
                # issue next step's loads before this step's store-waits so
                # store latency never delays input prefetch
                if t + 1 < T:
                    issue_x(t + 1)
                if t >= 1:
                    for j in range(2):
                        # store half as soon as its Sign completes
                        sync.wait_ge(s_a, sa_sign(t - 1, 1 + j))
                        sync.dma_start(
                            out=o_d[2 * j : 2 * j + 2, t - 1].rearrange(
                                "b c f -> c b f"
                            ),
                            in_=sl(ot[(t - 1) % 2], j).rearrange(
                                "p (b f) -> p b f", b=2
                            ),
                        ).then_inc(s_o, 16)
            # final step drains as 4 parallel per-batch stores
            for j in range(2):
                sync.wait_ge(s_a, sa_sign(T - 1, 1 + j))
                for b in range(2 * j, 2 * j + 2):
                    sync.dma_start(
                        out=o_d[b, T - 1],
                        in_=ot[(T - 1) % 2][:, b * HW : (b + 1) * HW],
                    ).then_inc(s_o, 16)
            sync.wait_ge(s_o, 32 * (T - 1) + 64)

        @block.vector
        def _(vector: bass.BassEngine):
            vector.memset(bm1[:, :], -1.0).then_inc(s_v, 1)
            for t in range(T):
                o = ot[t % 2]
                u = ud[t % 2] if t >= 1 else xt[0]
                if t >= 1:
                    # u'Dj = (v_Dj * 0.5) + x_Dj
                    for j in range(NSL):
                        vector.wait_ge(s_x[j], 64 + 32 * t)
                        if t >= 2:
                            # u slot free once SignDj(t-2) read it
                            vector.wait_ge(s_a, sa_sign(t - 2, 1 + j))
                        vector.scalar_tensor_tensor(
                            out=sl(u, j), in0=sl(vd, j), scalar=0.5,
                            in1=sl(xt[t % NX], j), op0=Alu.mult, op1=Alu.add,
                        ).then_inc(s_v, 1)
                if t <= T - 2:
                    # v_Dj = (o_Dj * -1) + u'Dj
                    for j in range(NSL):
                        vector.wait_ge(s_a, sa_sign(t, 1 + j))
                        vector.scalar_tensor_tensor(
                            out=sl(vd, j), in0=sl(o, j), scalar=-1.0,
                            in1=sl(u, j), op0=Alu.mult, op1=Alu.add,
                        ).then_inc(s_v, 1)

        @block.scalar
        def _(scalar: bass.BassEngine):
            for t in range(T):
                o = ot[t % 2]
                u = ud[t % 2] if t >= 1 else xt[0]
                for j in range(NSL):
                    if t == 0:
                        scalar.wait_ge(s_x[j], 64)
                        scalar.wait_ge(s_v, 1)  # bm1
                    else:
                        scalar.wait_ge(s_v, sv_u(t, 1 + j))
                    if t >= 2 and j == 0:
                        scalar.wait_ge(s_o, 32 * (t - 1))  # o slot stored
                        scalar.wait_ge(s_v, sv_r(t - 2, 2))  # o slot read
                    scalar.activation(
                        out=sl(o, j), in_=sl(u, j),
                        func=Act.Sign, bias=bm1[:, :], scale=1.0,
                    ).then_inc(s_a, 1)

    return nc


def _get_nc():
    if "nc" not in _cached:
        _cached["nc"] = _build_nc()
    return _cached["nc"]


def kernel(x_seq: np.ndarray) -> np.ndarray:
    import os

    from concourse.bass_utils import run_bass_kernel_spmd

    x = np.ascontiguousarray(np.asarray(x_seq, dtype=np.float32)).reshape(
        B, T, C, HW
    )
    nc = _get_nc()
    in_maps = [{"x": x[i * BLOC : (i + 1) * BLOC]} for i in range(NCORES)]
    trace = bool(os.environ.get("LIF_TRACE"))
    out = run_bass_kernel_spmd(nc, in_maps, list(range(NCORES)), trace=trace)
    _cached["last_results"] = out
    o = np.concatenate([r["o"] for r in out.results], axis=0)
    return o.reshape(B, T, C, 32, 32).astype(np.float32)
